# revision 66
# baseline (speedup 1.0000x reference)
"""DenseExpert MoE kernel for 8x Trainium2 NeuronCores (fp8 DoubleRow).

Math: r[b,u] = elu( sum_e g[b,e] * (x[b,:] @ alpha[e,u,:]) + (g @ beta)[b,u] )
Full shapes: x [4096,512] f32, g [4096,8] f32 (rows sum to 1),
alpha [8,512,512] f32, beta [8,512] f32 -> out [4096,512] f32.

Strategy: 2D shard over 8 cores = 4 batch shards x 2 U-column shards.
Per core: 1024 tokens x 256 output cols, all 8 experts.

Precision: fp8 e4m3 with a two-level (hi+lo) split of BOTH operands.
x~ = x*SX = Xhi + Xlo + eps_x, a~ = alpha*SA = Ahi + Alo + eps_a (each
level quantized to e4m3; residual quantization error ~0.13%).  The
product keeps 3 of 4 cross terms (drops Xlo*Alo ~ 0.13%): measured end
rel err ~1.0e-3 vs the 2e-2 gate.  All 3 products share the same
(SX*SA) scale so they accumulate in ONE f32 PSUM group; the 1/(SX*SA)
folds into the gate vector applied at combine time.

Each (expert, batch-tile) output [128,256] takes 6 DoubleRow matmuls
(K=256 per instruction via paired k-tiles, 0.5 cycles/col) instead of
4x2 bf16 matmuls (1 cycle/col) -- 0.75x the PE cycles -- and operand
DMA halves (fp8 bytes).  The g@beta bias is folded into expert FE's
PSUM group as a trailing K=8 matmul of host-prescaled gates
ghat[e,b] = g[b,e]*SX*SA/max(g[b,FE],eps), so the FE combine (scale
g[b,FE]/(SX*SA)) recovers g_FE*h_FE + g@beta exactly -- no separate
bias pass, and the tiny matmul is deferred until the gate tensor's
DMA lands so it never stalls the PE.

A tile's 6 matmuls take only ~320ns of PE but a gate-combine costs
~390ns on any one engine, so combines rotate over three paths
(pattern "dadp"): pure-DVE scalar_tensor_tensor from PSUM, ACT
copy+scale -> DVE fp16 add, and ACT copy+scale -> Pool add.  r
accumulates in fp16 SBUF (enables DVE 2x/4x fast modes for the ELU).
ELU = max(r, min(exp(r)-1, 0)) (exact): ACT exp + 2 fused fp16 DVE
ops (Pool variant uses add-based ops; gpsimd has no tensor max).
Outputs are stored fp16 (host upcasts) in batched group stores
(0,1)(2,3)(4,5,6)(7) so only one tiny store trails the final tile;
the drain phase walks expert-rows across bt4..7 so the four combine
chains advance in lockstep with the PE and the kernel ends on a
single short chain.

Host-side per core c (ib, iu = divmod(c, 2)), with Q = e4m3 quantize:
  xq[p, bt*8+lv*4+pr*2+kt, j] = Xlv[ib*1024+bt*128+j, (pr*2+kt)*128+p]
  aq[p, e*8+lv*4+pr*2+kt, u]  = Alv[e, iu*256+u, (pr*2+kt)*128+p]
  gb[e, :1024]  = g[.,e]*SX*SA/max(g[.,FE],1e-6)  (bf16; bias fold)
  gb[e, 1024:]  = beta[e, iu*256:(iu+1)*256]      (bf16)
  gs[p, bt, e]  = g[bt*128+p, e]/(SX*SA)  (e=FE: clamped like gb)
"""
import sys as _sys
for _p in ("/opt/trn_rl_repo", "/root/.axon_site/_ro/trn_rl_repo"):
    if _p not in _sys.path:
        _sys.path.append(_p)

import numpy as np
import ml_dtypes

N_CORES = 8
B, D, U, E = 4096, 512, 512, 8
BB, BU = 4, 2           # batch shards x u shards
BS = B // BB            # 1024 tokens per core
US = U // BU            # 256 output cols per core
BT = BS // 128          # 8 batch tiles per core
KT = D // 128           # 4 contraction blocks
NPAIR = 2               # DoubleRow k-pairs per D
LV = 2                  # hi/lo fp8 levels

SX, SA = 16.0, 64.0     # pre-quantization scales
G0EPS = 1e-6            # clamp for the bias-fold division
FE = 1                  # expert whose PSUM group carries the bias fold

N_WARM = 18             # junk PE matmuls to start the pstate ramp

# Combine paths: "d" = pure DVE scalar_tensor_tensor (392ns DVE),
# "a" = ACT copy+scale then DVE fp16 add (398 ACT + ~150 DVE),
# "p" = ACT copy+scale then Pool fp16 add (398 ACT + ~550 Pool).
# A tile's 6 matmuls take only 320ns of PE, so a single engine can't
# pace the combines -- rotate paths so consecutive tiles hit
# different engines (PSUM frees stay ahead of the matmul stream).
COMBINE_PATTERN = "dadp"

_CACHE = {}


def _build_module():
    import concourse.tile as tile
    from concourse import bacc, mybir

    f32 = mybir.dt.float32
    f16 = mybir.dt.float16
    bf16 = mybir.dt.bfloat16
    fp8 = mybir.dt.float8e4
    ADD = mybir.AluOpType.add
    MULT = mybir.AluOpType.mult
    MIN = mybir.AluOpType.min
    MAX = mybir.AluOpType.max
    Exp = mybir.ActivationFunctionType.Exp
    Copy = mybir.ActivationFunctionType.Copy
    DR = mybir.MatmulPerfMode.DoubleRow

    nc = bacc.Bacc("TRN2", target_bir_lowering=False, debug=False,
                   num_devices=N_CORES)
    xq_d = nc.dram_tensor("xq", [128, BT * LV * NPAIR * 2, 128], fp8,
                          kind="ExternalInput").ap()
    aq_d = nc.dram_tensor("aq", [128, E * LV * NPAIR * 2, US], fp8,
                          kind="ExternalInput").ap()
    # gb packs gh [E, BT*128] and beta [E, US] along the free dim
    gb_d = nc.dram_tensor("gb", [E, BT * 128 + US], bf16,
                          kind="ExternalInput").ap()
    gs_d = nc.dram_tensor("gs", [128, BT, E], f32,
                          kind="ExternalInput").ap()
    o_d = nc.dram_tensor("out", [BS, US], f16, kind="ExternalOutput").ap()

    with tile.TileContext(nc, trace_sim=True) as tc:
        with (
            tc.tile_pool(name="const", bufs=1) as cpool,
            tc.tile_pool(name="hps", bufs=7, space="PSUM") as hpool,
            tc.tile_pool(name="wps", bufs=1, space="PSUM") as wps,
            tc.tile_pool(name="rpool", bufs=BT) as rpool,
            tc.tile_pool(name="work", bufs=10) as wpool,
            tc.tile_pool(name="opool", bufs=4) as opool,
        ):
            # ---- tiles ----
            junk = cpool.tile([128, 256], bf16, tag="junk")
            gb = cpool.tile([E, BT * 128 + US], bf16, tag="gb")
            gs = cpool.tile([128, BT, E], f32, tag="gs")
            xq = cpool.tile([128, BT * LV * NPAIR * 2, 128], fp8, tag="xq")
            aq = cpool.tile([128, E * LV * NPAIR * 2, US], fp8, tag="aq")
            dump = cpool.tile([128, 8], f16, tag="dump")

            def ghT(bt):
                return gb[:, bt * 128:(bt + 1) * 128]

            beta_sb = gb[:, BT * 128:BT * 128 + US]

            # Pool memset starts ~immediately (DVE preamble is ~0.8us);
            # warm matmuls depend only on junk.
            nc.gpsimd.memset(junk[:], 0)
            # Dummy Exp pre-loads the ACT function table during ramp so
            # the first combine/ELU doesn't eat the 1.3us table load.
            nc.scalar.activation(dump[:], junk[:, 0:8], Exp)

            # ---- HWDGE loads (SP queue), supply order ----
            def load_a(e0, e1):
                nc.sync.dma_start(aq[:, e0 * 8:e1 * 8, :],
                                  aq_d[:, e0 * 8:e1 * 8, :])

            def load_x(b0, b1):
                nc.sync.dma_start(xq[:, b0 * 8:b1 * 8, :],
                                  xq_d[:, b0 * 8:b1 * 8, :])

            load_a(0, 1)
            load_x(0, 2)
            load_a(1, 2)
            load_x(2, 4)
            nc.sync.dma_start(gs[:], gs_d[:])
            nc.sync.dma_start(gb[:], gb_d[:])
            load_a(2, 3)
            load_x(4, 6)
            load_a(3, 4)
            load_x(6, 8)
            load_a(4, 6)
            load_a(6, 8)

            # ---- PE warm-up matmuls (junk data, starts pstate ramp) ----
            # One PSUM tile, same-engine WAW => program order, no sems.
            warm_ps = wps.tile([128, 256], f32, tag="w", name="warm")
            for w in range(N_WARM):
                nc.tensor.matmul(warm_ps[:], lhsT=junk[:, 0:128],
                                 rhs=junk[:], start=True, stop=True,
                                 skip_group_check=True)

            # ---- main compute pieces ----
            r_sbs = [rpool.tile([128, US], f16, tag="r", name=f"r{bt}")
                     for bt in range(BT)]

            def dr_mms(e, bt, ps, u0=0, u1=US, close=True):
                # 6 DoubleRow fp8 matmuls: products (xlv,alv) in
                # {hi*hi, lo*hi, hi*lo}, each over 2 k-pairs.  For e==FE
                # the g@beta bias joins the group as a trailing K=8
                # matmul of host-prescaled gates (see core_inputs),
                # possibly deferred until gb arrives (close=False).
                first = True
                for xlv, alv in ((0, 0), (1, 0), (0, 1)):
                    for pr in range(NPAIR):
                        xi = bt * 8 + xlv * 4 + pr * 2
                        ai = e * 8 + alv * 4 + pr * 2
                        last = (xlv, alv, pr) == (0, 1, NPAIR - 1)
                        nc.tensor.matmul(
                            ps[:], lhsT=xq[:, xi:xi + 2, :],
                            rhs=aq[:, ai:ai + 2, u0:u1],
                            start=first, stop=(last and e != FE),
                            perf_mode=DR, skip_group_check=True)
                        first = False
                if e == FE and close:
                    nc.tensor.matmul(ps[:], lhsT=ghT(bt),
                                     rhs=beta_sb[:, u0:u1], start=False,
                                     stop=True, skip_group_check=True)

            ctile = [0]

            def combine(e, bt, ps, path=None):
                r = r_sbs[bt]
                if e == 0:
                    nc.scalar.activation(r[:], ps[:], Copy,
                                         scale=gs[:, bt, 0:1])
                    return
                if path is None:
                    path = COMBINE_PATTERN[ctile[0] % len(COMBINE_PATTERN)]
                    ctile[0] += 1
                if path == "d":
                    nc.vector.scalar_tensor_tensor(
                        out=r[:], in0=ps[:],
                        scalar=gs[:, bt, e:e + 1],
                        in1=r[:], op0=MULT, op1=ADD)
                else:
                    t = wpool.tile([128, US], f16, tag="ct",
                                   name=f"ct_{e}_{bt}")
                    nc.scalar.activation(t[:], ps[:], Copy,
                                         scale=gs[:, bt, e:e + 1])
                    eng = nc.gpsimd if path == "p" else nc.vector
                    eng.tensor_tensor(r[:], r[:], t[:], ADD)

            fe_open = {}

            def mm(e, bt, defer_close=False, path=None):
                if (e, bt) in fe_open:
                    ps = fe_open.pop((e, bt))
                    nc.tensor.matmul(ps[:], lhsT=ghT(bt), rhs=beta_sb[:],
                                     start=False, stop=True,
                                     skip_group_check=True)
                else:
                    ps = hpool.tile([128, US], f32, tag="h",
                                    name=f"h_{e}_{bt}")
                    dr_mms(e, bt, ps, close=not defer_close)
                    if defer_close:
                        fe_open[(e, bt)] = ps
                        return
                combine(e, bt, ps, path=path)

            o_view = o_d.rearrange("(bt p) u -> p bt u", p=128)
            # store groups: (0,1) (2,3) (4,5,6) (7) -- the last store is
            # tiny so the fixed store pipe after the final ELU is short
            OGROUP = {0: (0, 0), 1: (0, 1), 2: (1, 0), 3: (1, 1),
                      4: (2, 0), 5: (2, 1), 6: (2, 2), 7: (3, 0)}
            OBASE = [0, 2, 4, 7]
            OLEN = [2, 2, 3, 1]
            otiles = [opool.tile([128, n, US], f16, tag="op",
                                 name=f"op{p}")
                      for p, n in enumerate(OLEN)]

            def oslot(bt):
                gidx, slot = OGROUP[bt]
                return otiles[gidx][:, slot, :]

            def elu(bt, pool=False):
                # ELU = max(r, min(exp(r)-1, 0)) (exact); fp16 fused ops
                # keep the DVE 2x/4x fast modes in play.  Output lands in
                # the bt-group tile; one batched store per group.
                # pool=True keeps the DVE queue clear of exp-dependent
                # waits during the drain.
                r = r_sbs[bt]
                t = wpool.tile([128, US], f16, tag="t", name=f"t_{bt}")
                nc.scalar.activation(t[:], r[:], Exp)
                m = wpool.tile([128, US], f16, tag="m", name=f"m_{bt}")
                if pool:
                    # GPSIMD has no tensor_tensor(max) kernel: build
                    # relu(r) + min(t-1, 0) from add-based pool ops.
                    nc.gpsimd.tensor_scalar(out=m[:], in0=t[:],
                                            scalar1=-1.0, scalar2=0.0,
                                            op0=ADD, op1=MIN)
                    p = wpool.tile([128, US], f16, tag="p", name=f"p_{bt}")
                    nc.gpsimd.tensor_scalar(out=p[:], in0=r[:],
                                            scalar1=0.0, scalar2=0.0,
                                            op0=MAX, op1=ADD)
                    nc.gpsimd.tensor_tensor(oslot(bt), p[:], m[:], ADD)
                else:
                    nc.vector.tensor_scalar(out=m[:], in0=t[:],
                                            scalar1=-1.0, scalar2=0.0,
                                            op0=ADD, op1=MIN)
                    nc.vector.tensor_tensor(oslot(bt), r[:], m[:], MAX)

            def store_group(p, q=None):
                (q or nc.sync).dma_start(
                    o_view[:, OBASE[p]:OBASE[p] + OLEN[p], :],
                    otiles[p][:])

            # ---- schedule: staircase matched to load arrival order ----
            # (1,0)/(1,1) groups stay open until gb lands (bias close)
            mm(0, 0)
            mm(0, 1)
            mm(1, 0, defer_close=True)
            mm(1, 1, defer_close=True)
            mm(0, 2)
            mm(0, 3)
            mm(1, 0)
            mm(1, 1)
            mm(1, 2)
            mm(1, 3)
            mm(2, 0)
            mm(2, 1)
            mm(2, 2)
            mm(2, 3)
            mm(0, 4)
            mm(0, 5)
            mm(1, 4)
            mm(1, 5)
            mm(3, 0)
            mm(3, 1)
            mm(3, 2)
            mm(3, 3)
            mm(0, 6)
            mm(0, 7)
            mm(1, 6)
            mm(1, 7)
            mm(2, 4)
            mm(2, 5)
            mm(3, 4)
            mm(3, 5)
            mm(2, 6)
            mm(2, 7)
            mm(3, 6)
            mm(3, 7)
            # finishing phase for bt0..3: bt-pairs; ELU drains while
            # later tiles matmul
            for b0 in (0, 2):
                for e in range(4, E):
                    mm(e, b0)
                    mm(e, b0 + 1)
                elu(b0)
                elu(b0 + 1)
                store_group(b0 // 2)
            # drain phase bt4..7: expert-rows so all four combine chains
            # advance one step per row; the final tile is column-split
            # and the last store is a single tiny tile.
            for e in (4, 5):
                for bt in (4, 5, 6, 7):
                    mm(e, bt)
            mm(6, 4)
            mm(6, 5)
            mm(7, 4, path="d")
            mm(6, 6)
            mm(7, 5, path="d")
            mm(6, 7)
            elu(4, pool=True)
            mm(7, 6, path="d")
            mm(7, 7, path="d")
            elu(5)
            elu(6)
            store_group(2)
            elu(7)
            store_group(3, q=nc.scalar)
    nc.compile()
    return nc


def get_module():
    if "nc" not in _CACHE:
        _CACHE["nc"] = _build_module()
    return _CACHE["nc"]


def _q8(a):
    return a.astype(ml_dtypes.float8_e4m3)


def core_inputs(inputs, c):
    """Host-side packing of FULL inputs into core c's tile layouts."""
    f8 = ml_dtypes.float8_e4m3
    ib, iu = divmod(c, BU)
    x_c = inputs["x"][ib * BS:(ib + 1) * BS]            # [BS, D]
    g_c = inputs["g"][ib * BS:(ib + 1) * BS]            # [BS, E]
    a_c = inputs["alpha"][:, iu * US:(iu + 1) * US, :]  # [E, US, D]
    b_c = inputs["beta"][:, iu * US:(iu + 1) * US]      # [E, US]

    # two-level e4m3 split of x*SX and alpha*SA
    xs = x_c.astype(np.float32) * SX
    xhi = _q8(xs)
    xlo = _q8(xs - xhi.astype(np.float32))
    asc = a_c.astype(np.float32) * SA
    ahi = _q8(asc)
    alo = _q8(asc - ahi.astype(np.float32))

    # xq[p, bt*8 + lv*4 + pr*2 + kt, j] = Xlv[bt*128+j, (pr*2+kt)*128+p]
    def pack_x(xl):
        v = xl.reshape(BT, 128, NPAIR, 2, 128)   # [bt, j, pr, kt, p]
        return v.transpose(4, 0, 2, 3, 1)        # [p, bt, pr, kt, j]
    xq = np.stack([pack_x(xhi), pack_x(xlo)], axis=2)  # [p,bt,lv,pr,kt,j]
    xq = np.ascontiguousarray(
        xq.reshape(128, BT * LV * NPAIR * 2, 128)).astype(f8)

    # aq[p, e*8 + lv*4 + pr*2 + kt, u] = Alv[e, u, (pr*2+kt)*128+p]
    def pack_a(al):
        v = al.reshape(E, US, NPAIR, 2, 128)     # [e, u, pr, kt, p]
        return v.transpose(4, 0, 2, 3, 1)        # [p, e, pr, kt, u]
    aq = np.stack([pack_a(ahi), pack_a(alo)], axis=2)  # [p,e,lv,pr,kt,u]
    aq = np.ascontiguousarray(
        aq.reshape(128, E * LV * NPAIR * 2, US)).astype(f8)

    # bias-fold gates gh[e, bt*128+j], combine gates gs[p, bt, e], and
    # beta -- gh and beta pack into one [E, BT*128+US] bf16 tensor.
    # The bias rides in expert FE's PSUM group, scaled by SX*SA/g_FE so
    # the FE combine (scale g_FE/(SX*SA)) recovers g@beta exactly.
    gf = np.maximum(g_c[:, FE:FE + 1], G0EPS)           # [BS, 1]
    gh = (g_c * (SX * SA) / gf).T                       # [E, BS]
    gb = np.concatenate([gh.astype(np.float32), b_c], axis=1)
    gb = np.ascontiguousarray(gb).astype(ml_dtypes.bfloat16)
    gsv = g_c.astype(np.float32) / (SX * SA)            # [BS, E]
    gsv = gsv.copy()
    gsv[:, FE] = gf[:, 0] / (SX * SA)
    gs = np.ascontiguousarray(gsv.reshape(BT, 128, E).transpose(1, 0, 2))

    return {
        "xq": xq,
        "aq": aq,
        "gb": gb,
        "gs": gs.astype(np.float32),
    }


def kernel(x, g, alpha, beta):
    from concourse.bass_utils import run_bass_kernel_spmd

    nc = get_module()
    inputs = {
        "x": np.ascontiguousarray(x, dtype=np.float32),
        "g": np.ascontiguousarray(g, dtype=np.float32),
        "alpha": np.ascontiguousarray(alpha, dtype=np.float32),
        "beta": np.ascontiguousarray(beta, dtype=np.float32),
    }
    in_maps = [core_inputs(inputs, c) for c in range(N_CORES)]
    res = run_bass_kernel_spmd(nc, in_maps, list(range(N_CORES)))
    out = np.empty((B, U), dtype=np.float32)
    for c in range(N_CORES):
        ib, iu = divmod(c, BU)
        out[ib * BS:(ib + 1) * BS, iu * US:(iu + 1) * US] = \
            res.results[c]["out"].astype(np.float32)
    return out
